# revision 19
# baseline (speedup 1.0000x reference)
"""DGCNN (SGConv K=2 + conv-pool + fc) Trainium2 kernel.

Math:
  A_norm = D^-1/2 (A + I) D^-1/2   (A from tril edge_w, symmetrized)
  h      = relu(A_norm^2 @ x @ lin_w + lin_b)        [B, N, H]
  pooled = relu(einsum('bnh,n->bh', h, conv_w) + conv_b)
  out    = pooled @ fc_w + fc_b                      [B, C]

Device strategy (data-parallel over batch, 8 cores x 512 batches):
  Host folds the two SGConv hops into A2 = A_norm @ A_norm and folds the
  SIGNED conv weight c into A2's columns, so the A2-hop matmul outputs
  w[i] = c_i * z2[i].  With s = sign(c):
      pooled = sum_i s_i relu(|c_i| z2_i) = 1/2 (sum_i w_i + sum_i s_i |w_i|)
  The first term is a LINEAR functional of x -> host computes it directly
  as (x . q) @ lin_w with q = A2 @ c (one cheap numpy pass).  The second
  term is two abs-valued free-dim reduces (A2 columns are permuted
  pos-signs-first) read straight from PSUM -- no relu, no pooling matmul.

  Per 16-batch iteration (fp16 operands; PSUM fp32):
    PE : MM_L x16: z[j, bh]  = x_b @ lin_w     (lhsT = xT_b slice)
         MM_A x8 : w[bh, i]  = z_pair^T @ A2c  (lhsT = z 2-batch block)
    ACT: z PSUM -> SBUF fp16 (feeds MM_A's stationary operand)
    DVE: tensor_reduce(|.|) over i in [0,npos) and [npos,128) -> PP cols
  One 256KB DMA ships PP at the end; x streams in 2 MB chunks (sync ring).
  Measured 58-60 us/core (from 84 us baseline); rel-err ~8.7e-4.

  Host epilogue: pooled = (wsum + pos - neg)/2; relu(pooled + conv_b) @ fc_w.
"""

import ml_dtypes
import numpy as np

import concourse.bacc as bacc
import concourse.bass as bass
import concourse.mybir as mybir
import concourse.tile as tile
from concourse.bass_utils import run_bass_kernel_spmd

N = 128       # nodes
F_IN = 128    # in features
H = 64        # hidden
C = 40        # classes
B = 4096      # batch
NCORES = 8
BPC = B // NCORES          # 512 batches per core
G = 16                     # batches per iteration (2 PSUM banks: 16*64 fp32)
NPAIR = G // 2             # 2-batch pairs per iteration
NG = BPC // G              # 32 iterations
CHUNK = 4                  # iterations per x DMA (4*16 batches = 2 MB)
NCHUNK = NG // CHUNK
# chunk 0 piece list (measured: splitting chunk 0 hurts; keep single)
C0_PIECES = [(0, CHUNK)]
ZS_BUFS = 4
NQ = BPC // 2              # total 2-batch pairs per core (PP columns)

F32 = mybir.dt.float32
BF16 = mybir.dt.bfloat16
FP16 = mybir.dt.float16
RELU = mybir.ActivationFunctionType.Relu
AXIS_X = mybir.AxisListType.X
ADD = mybir.AluOpType.add

MM_DT = FP16
X_DT = FP16

_PROG_CACHE: dict = {}
_last_in_maps: list = []
_NPOS = 57   # baked reduce split; set from conv_w before building

# ablation: 'full', 'no_r' (skip reduces), 'lin_only' (MM_L + copy),
# 'dma_only'
_VARIANT = "full"
DMA_ALT = True


def _build_program(has_bias: bool, repeat: int = 1):
    npos = _NPOS
    nc = bacc.Bacc(
        "TRN2", target_bir_lowering=False, debug=False, num_devices=NCORES
    )
    xP = nc.declare_dram_parameter(
        "xP", [NCHUNK, F_IN, CHUNK * G, N], X_DT, isOutput=False
    )
    a2c = nc.declare_dram_parameter("a2c", [N, N], MM_DT, isOutput=False)
    linw = nc.declare_dram_parameter("linw", [F_IN, H], X_DT, isOutput=False)
    if has_bias:
        btile = nc.declare_dram_parameter("btile", [N, NPAIR * N], F32, isOutput=False)
    # [:, 0:NQ] = abs-sums over pos-sign block, [:, NQ:2NQ] = neg block
    pooledpn = nc.declare_dram_parameter("pooledpn", [N, 2 * NQ], F32, isOutput=True)

    with tile.TileContext(nc) as tc:
        with (
            tc.tile_pool(name="const", bufs=1) as constp,
            tc.tile_pool(name="xin", bufs=3) as xinp,
            tc.tile_pool(name="zs", bufs=ZS_BUFS) as zsp,
            tc.tile_pool(name="ppp", bufs=2) as ppp,
            tc.tile_pool(name="psL", bufs=2, space="PSUM") as psL,
            tc.tile_pool(name="psA", bufs=4, space="PSUM") as psA,
        ):
            a2c_t = constp.tile([N, N], MM_DT)
            nc.sync.dma_start(a2c_t[:], a2c[:, :])
            linw_t = constp.tile([F_IN, H], X_DT)
            nc.sync.dma_start(linw_t[:], linw[:, :])
            if has_bias:
                bt_t = constp.tile([N, NPAIR * N], F32)
                nc.sync.dma_start(bt_t[:], btile[:, :])

            import contextlib

            loop_cm = (
                tc.For_i(0, repeat, 1) if repeat > 1 else contextlib.nullcontext()
            )

            with loop_cm:
                PP = ppp.tile([N, 2 * NQ], F32, name="PP", tag="PP")

                zst_q: dict = {}
                zps_q: dict = {}
                ups_q: dict = {}
                X_cur: list = [None]

                def stage_L(i):
                    if i % CHUNK == 0:
                        c = i // CHUNK
                        X8 = xinp.tile(
                            [F_IN, CHUNK * G * N], X_DT, name="X8", tag="X"
                        )
                        pieces = C0_PIECES if c == 0 else [(0, CHUNK)]
                        for p0, plen in pieces:
                            nc.sync.dma_start(
                                X8[:, p0 * G * N : (p0 + plen) * G * N].rearrange(
                                    "p (b j) -> p b j", b=plen * G
                                ),
                                xP[c, :, p0 * G : (p0 + plen) * G],
                            )
                        X_cur[0] = X8
                    X = X_cur[0]
                    off = (i % CHUNK) * G * N
                    zps = psL.tile([N, G * H], F32, tag="zps")
                    zps_q[i] = zps
                    zst = zsp.tile([N, G * H], MM_DT, tag="zst")
                    zst_q[i] = zst

                HP = NPAIR // 2  # pairs per PSUM-bank half

                def stage_Lmm(i, half):
                    X = X_cur[0] if i % CHUNK == 0 else X_cur[0]
                    X = X_cur[0]
                    off = (i % CHUNK) * G * N
                    zps = zps_q[i]
                    h0 = half * (G // 2)
                    for b in range(h0, h0 + G // 2):
                        nc.tensor.matmul(
                            zps[:, b * H : (b + 1) * H],
                            lhsT=X[:, off + b * N : off + (b + 1) * N],
                            rhs=linw_t[:],
                            start=True,
                            stop=True,
                        )
                    zst = zst_q[i]
                    c0 = half * (G * H // 2)
                    c1 = c0 + G * H // 2
                    nc.scalar.copy(zst[:, c0:c1], zps[:, c0:c1])
                    if half == 1:
                        zps_q.pop(i)

                def stage_A(i, half):
                    zst = zst_q[i]
                    if half == 1:
                        zst_q.pop(i)
                    ups = psA.tile([N, HP * N], F32, tag="ups")
                    for p in range(HP):
                        pp_ = half * HP + p
                        nc.tensor.matmul(
                            ups[:, p * N : (p + 1) * N],
                            lhsT=zst[:, pp_ * N : (pp_ + 1) * N],
                            rhs=a2c_t[:],
                            start=True,
                            stop=True,
                        )
                    ups_q[(i, half)] = ups

                def stage_R(i, half):
                    ups = ups_q.pop((i, half))
                    if has_bias:
                        wb = zsp.tile([N, HP * N], F32, tag="wb")
                        nc.vector.tensor_add(
                            wb[:], ups[:], bt_t[:, half * HP * N : (half + 1) * HP * N]
                        )
                        src = wb
                    else:
                        src = ups
                    u3 = src[:].rearrange("m (q i) -> m q i", q=HP)
                    col = i * NPAIR + half * HP
                    if npos > 0:
                        nc.vector.tensor_reduce(
                            PP[:, col : col + HP],
                            u3[:, :, 0:npos],
                            axis=AXIS_X,
                            op=ADD,
                            apply_absolute_value=True,
                        )
                    if npos < N:
                        nc.vector.tensor_reduce(
                            PP[:, NQ + col : NQ + col + HP],
                            u3[:, :, npos:N],
                            axis=AXIS_X,
                            op=ADD,
                            apply_absolute_value=True,
                        )

                if _VARIANT == "dma_only":
                    for c in range(NCHUNK):
                        X8 = xinp.tile(
                            [F_IN, CHUNK * G * N], X_DT, name="X8d", tag="X"
                        )
                        nc.sync.dma_start(
                            X8[:].rearrange("p (b j) -> p b j", b=CHUNK * G),
                            xP[c],
                        )
                        if c == NCHUNK - 1:
                            nc.vector.tensor_copy(
                                PP[:, 0:128], X8[:, 0:256].bitcast(F32)
                            )
                else:
                    run_A = _VARIANT in ("full", "no_r")
                    run_R = _VARIANT == "full"
                    # Engine queue order: reduces (oldest deps) before this
                    # iteration's matmuls/copies so a wait on fresh data never
                    # blocks ready work behind it.
                    for i in range(NG + 2):
                        if i >= 2 and i - 2 < NG and run_R:
                            stage_R(i - 2, 0)
                            stage_R(i - 2, 1)
                        if _VARIANT == "no_r" and 2 <= i < NG + 2:
                            ups_q.pop((i - 2, 0))
                            ups_q.pop((i - 2, 1))
                        if i < NG:
                            stage_L(i)
                            stage_Lmm(i, 0)
                        if 1 <= i < NG + 1 and run_A:
                            stage_A(i - 1, 0)
                        if i < NG:
                            stage_Lmm(i, 1)
                        if 1 <= i < NG + 1 and run_A:
                            stage_A(i - 1, 1)
                    if _VARIANT == "lin_only":
                        for k in list(zst_q):
                            zst_q.pop(k)
                    if _VARIANT in ("lin_only", "no_r"):
                        nc.vector.memset(PP[:, 0 : 2 * NQ], 0.0)

                nc.sync.dma_start(pooledpn[:, :], PP[:])
    nc.compile()
    return nc


def _get_program(has_bias: bool):
    key = (has_bias, MM_DT, _NPOS, _VARIANT, DMA_ALT)
    if key not in _PROG_CACHE:
        _PROG_CACHE[key] = _build_program(has_bias)
    return _PROG_CACHE[key]


def _host_adjacency(edge_w, conv_w):
    """A2 with signed c folded into columns, permuted pos-sign-first; and
    q = A2 @ c for the host-side linear term."""
    ew = np.asarray(edge_w, dtype=np.float64)
    A = np.zeros((N, N), dtype=np.float64)
    xs, ys = np.tril_indices(N)
    A[xs, ys] = ew
    A = A + A.T - np.diag(np.diag(A))
    Ah = A + np.eye(N)
    deg = Ah.sum(axis=1)
    dinv = np.where(deg > 0, deg ** -0.5, 0.0)
    An = dinv[:, None] * Ah * dinv[None, :]
    A2 = An @ An
    c = np.asarray(conv_w, dtype=np.float64)
    a2cs = A2 * c[None, :]              # a2cs[j, i] = A2[j, i] * c_i
    q = A2 @ c                          # q_j = sum_i A2[j, i] c_i
    s = np.sign(c)
    perm = np.concatenate([np.where(s > 0)[0], np.where(s <= 0)[0]])
    npos = int((s > 0).sum())
    a2cp = np.ascontiguousarray(a2cs[:, perm]).astype(np.float32)
    return a2cp, q, perm, npos


def _run(inputs: dict, trace: bool = False):
    global _NPOS
    x = np.asarray(inputs["x"], dtype=np.float32)
    edge_w = np.asarray(inputs["edge_w"], dtype=np.float32)
    lin_w = np.ascontiguousarray(np.asarray(inputs["lin_w"], dtype=np.float32))
    lin_b = np.asarray(inputs["lin_b"], dtype=np.float32)
    conv_w = np.asarray(inputs["conv_w"], dtype=np.float32)
    conv_b = np.asarray(inputs["conv_b"], dtype=np.float32)
    fc_w = np.asarray(inputs["fc_w"], dtype=np.float32)
    fc_b = np.asarray(inputs["fc_b"], dtype=np.float32)

    a2cp, q, perm, npos = _host_adjacency(edge_w, conv_w)
    _NPOS = npos
    has_bias = bool(np.any(lin_b != 0))
    nc = _get_program(has_bias)

    _np_of = {F32: np.float32, BF16: ml_dtypes.bfloat16, FP16: np.float16}
    np_xdt = _np_of[X_DT]
    np_mmdt = _np_of[MM_DT]
    linw_dev = lin_w.astype(np_xdt)
    a2cp_dev = a2cp.astype(np_mmdt)
    in_maps = []
    for k in range(NCORES):
        xc = x[k * BPC : (k + 1) * BPC]                  # [512, j, f]
        xc = xc.reshape(NCHUNK, CHUNK * G, N, F_IN)      # [c, b, j, f]
        xPk = np.ascontiguousarray(
            xc.transpose(0, 3, 1, 2).astype(np_xdt)
        )  # [c, f, b, j]
        m = {"xP": xPk, "a2c": a2cp_dev, "linw": linw_dev}
        if has_bias:
            # bias term in [m=(b2,h), (p,i)] layout: c_perm(i) * lin_b[h]
            cp = np.asarray(conv_w, dtype=np.float64)[perm]
            col = np.tile(cp, NPAIR)                      # [(p, i)]
            row = np.tile(lin_b.astype(np.float64), 2)    # [m]
            m["btile"] = np.ascontiguousarray(
                np.outer(row, col).astype(np.float32)
            )
        in_maps.append(m)

    global _last_in_maps
    _last_in_maps = in_maps
    try:
        res = run_bass_kernel_spmd(nc, in_maps, list(range(NCORES)), trace=trace)
    except ModuleNotFoundError:
        res = run_bass_kernel_spmd(nc, in_maps, list(range(NCORES)), trace=False)

    # Host linear term: wsum[b, h] = sum_j q_j z[b, j, h] = ((x . q) @ lin_w)
    y = np.tensordot(x, q.astype(np.float32), axes=([1], [0]))   # [B, F]
    wsum = y @ lin_w                                             # [B, H]
    if has_bias:
        wsum = wsum + float(np.sum(conv_w.astype(np.float64))) * lin_b[None, :]

    # unpack: PP[m, q] m=(b2*64+h), q=global pair; pooled[2q+b2, h]
    pooled_parts = []
    for k in range(NCORES):
        pp = res.results[k]["pooledpn"].astype(np.float64)       # [128, 2NQ]
        pos = pp[:, 0:NQ] if npos > 0 else 0.0
        neg = pp[:, NQ : 2 * NQ] if npos < N else 0.0
        sabs = pos - neg                                          # [128, NQ]
        pooled_parts.append(
            np.asarray(sabs).reshape(2, H, NQ).transpose(2, 0, 1).reshape(BPC, H)
        )
    sabs_all = np.concatenate(pooled_parts, axis=0)               # [B, H]
    pooled = 0.5 * (wsum.astype(np.float64) + sabs_all)

    p = np.maximum(pooled + conv_b[0], 0.0).astype(np.float32)
    out = (p @ fc_w + fc_b).astype(np.float32)
    return out, res


def kernel(x, edge_w, lin_w, lin_b, conv_w, conv_b, fc_w, fc_b):
    out, _ = _run(
        {
            "x": x,
            "edge_w": edge_w,
            "lin_w": lin_w,
            "lin_b": lin_b,
            "conv_w": conv_w,
            "conv_b": conv_b,
            "fc_w": fc_w,
            "fc_b": fc_b,
        }
    )
    return out
